# revision 1
# baseline (speedup 1.0000x reference)
"""Depthwise causal Conv1d (K=16) for x:(4, 2048, 8192) f32 on 8 TRN2 NeuronCores.

Strategy (tensor-parallel over channels, no cross-core communication):
  - Each core owns 256 channels (2048 / 8) for all 4 batches.
  - The time axis is cut into overlapping 127-sample windows with stride 112
    (15-sample causal halo), placed on SBUF partitions 0..126 and
    time-REVERSED within each window; partition 127 carries a constant 1.0
    row.  The depthwise conv (+bias) of one channel is then a single
    banded-Toeplitz matmul on the TensorEngine:
        psum[m, (b,j)] = sum_p A[p, m] * X[p, (b,j)]
        A[p, m]   = w[126 - p - m]            for 111 <= p + m <= 126
        A[127, m] = bias_c                    (ones-row of X -> +bias)
        X[p, (b,j)] = x[b, c, 112*j + 111 - p]  (zero outside [0, T))
        X[127, (b,j)] = 1.0
        psum[m, (b,j)] = y[b, c, 112*j + m]     for m < 112
    The reversal makes every host/device access pattern a purely
    positive-stride AP; the halo removes any cross-window boundary matmul;
    112 = 7*16 output rows keep the store DMAs port-balanced.
  - Band matrices A (56.5 KiB/channel) are built on the host and shipped in
    a p-major layout so each DMA descriptor is a large contiguous run.
  - Epilogue: pure PSUM -> SBUF copies, two channels per instruction
    (channel pair in one 2-bank PSUM tile), alternating Vector / Scalar
    engines, then one large store per chunk.

The host does the sharding + window-layout transposes with numpy; the device
kernel sees only dense p-major arrays.
"""

import os
import sys

import numpy as np
from numpy.lib.stride_tricks import sliding_window_view

if "/opt/trn_rl_repo" not in sys.path:
    sys.path.insert(0, "/opt/trn_rl_repo")

import concourse.bacc as bacc
import concourse.mybir as mybir
import concourse.tile as tile
from concourse.bass_utils import run_bass_kernel_spmd

F32 = mybir.dt.float32
F32R = mybir.dt.float32r
ACT_COPY = mybir.ActivationFunctionType.Copy

N_CORES = 8
B = 4             # batch
DIM = 2048        # channels
T = 8192          # time
K = 16            # conv taps
C = DIM // N_CORES    # channels per core = 256
PIN = 128         # matmul contraction rows (127 x-samples + ones row)
PO = 112          # outputs per window (= 127 - 15), multiple of 16
NJ = -(-T // PO)      # windows per (batch, channel) = 74
XC = B * NJ           # x / out cols per channel = 296
PSB = 512             # psum bank stride (f32 elems); channel pair at 0 / PSB
CH = 8            # channels per device chunk
NCHUNK = C // CH  # 16

# matmul dtype: float32 (exact, 4 cyc/row) or float32r (fast, ~1e-4 rel err)
MM_DTYPE = F32R if os.environ.get("CONV_MM_F32R", "0") == "1" else F32

_compiled_nc = None


def _build_kernel():
    nc = bacc.Bacc(None)

    xin = nc.declare_dram_parameter("xin", [PIN, C, XC], MM_DTYPE, isOutput=False)
    a_in = nc.declare_dram_parameter("a_in", [PIN, C, PO], MM_DTYPE, isOutput=False)
    yout = nc.declare_dram_parameter("yout", [PO, C, XC], F32, isOutput=True)

    ablate = os.environ.get("CONV_ABLATE", "") == "dmaonly"

    with tile.TileContext(nc) as tc:
        with (
            tc.tile_pool(name="xpool", bufs=9) as xpool,
            tc.tile_pool(name="apool", bufs=7) as apool,
            tc.tile_pool(name="opool", bufs=7) as opool,
            tc.tile_pool(name="psum", bufs=4, space="PSUM") as pspool,
        ):
            for chunk in range(NCHUNK):
                c0 = chunk * CH
                x_t = xpool.tile([PIN, CH * XC], MM_DTYPE)
                a_t = apool.tile([PIN, CH * PO], MM_DTYPE)
                o_t = opool.tile([PO, CH * XC], F32)

                nc.gpsimd.dma_start(
                    out=x_t[:].rearrange("p (c j) -> p c j", c=CH),
                    in_=xin[:, c0 : c0 + CH, :],
                )
                a_eng = nc.scalar if chunk % 2 == 0 else nc.sync
                a_eng.dma_start(
                    out=a_t[:].rearrange("p (c m) -> p c m", c=CH),
                    in_=a_in[:, c0 : c0 + CH, :],
                )

                if ablate:
                    nc.vector.tensor_copy(
                        o_t[:, 0:XC], x_t[0:PO, 0:XC].bitcast(F32)
                    )
                else:
                    for g in range(CH // 2):
                        ps = pspool.tile([PO, 2 * PSB], F32)
                        for h in range(2):
                            i = 2 * g + h
                            nc.tensor.matmul(
                                ps[:, h * PSB : h * PSB + XC],
                                a_t[:, i * PO : (i + 1) * PO],
                                x_t[:, i * XC : (i + 1) * XC],
                                start=True,
                                stop=True,
                            )
                        # pure psum -> sbuf copy, 2 channels per instruction
                        src = ps[:].rearrange("p (g q) -> p g q", g=2)[:, :, 0:XC]
                        dst = o_t[:, 2 * g * XC : (2 * g + 2) * XC].rearrange(
                            "p (g q) -> p g q", g=2
                        )
                        if g % 2 == 0:
                            nc.vector.tensor_copy(dst, src)
                        else:
                            nc.scalar.activation(dst, src, ACT_COPY)

                s_eng = nc.sync if chunk % 2 == 0 else nc.scalar
                s_eng.dma_start(
                    out=yout[:, c0 : c0 + CH, :],
                    in_=o_t[:].rearrange("p (c j) -> p c j", c=CH),
                )

    nc.compile()
    return nc


def _get_nc():
    global _compiled_nc
    if _compiled_nc is None:
        _compiled_nc = _build_kernel()
    return _compiled_nc


def _prep_core(x, weight, bias, core):
    """Build the per-core input map (numpy only)."""
    cs = slice(core * C, (core + 1) * C)
    xs = x[:, cs, :]                       # [B, C, T]
    w = weight[cs, 0, :]                   # [C, K]
    bs = bias[cs]                          # [C]

    # X[p, c, (b, j)] = xpad[b, c, 112*j + 126 - p] for p < 127; X[127] = 1
    # xpad = [15 zeros] ++ x ++ [right pad]
    xpad = np.zeros((B, C, PO * (NJ - 1) + PIN - 1), dtype=np.float32)
    xpad[:, :, K - 1 : K - 1 + T] = xs
    sw = sliding_window_view(xpad, PIN - 1, axis=2)[:, :, :: PO, :]  # [B,C,NJ,127]
    xin = np.empty((PIN, C, B, NJ), dtype=np.float32)
    xin[0 : PIN - 1] = sw[:, :, :, ::-1].transpose(3, 1, 0, 2)
    xin[PIN - 1] = 1.0
    xin = np.ascontiguousarray(xin).reshape(PIN, C, XC)

    # A[p, m] = w[126 - p - m] for 111 <= p + m <= 126; A[127, m] = bias
    idx = np.arange(PIN - 1)[:, None] + np.arange(PO)[None, :]   # p + m
    amask = (idx >= 111) & (idx <= 126)
    aidx = np.clip(126 - idx, 0, K - 1)
    a_mat = np.where(amask[None], w[:, aidx], 0.0)               # [C, 127, PO]
    a_in = np.empty((PIN, C, PO), dtype=np.float32)
    a_in[0 : PIN - 1] = a_mat.transpose(1, 0, 2)
    a_in[PIN - 1] = bs[:, None]
    a_in = np.ascontiguousarray(a_in)

    return {"xin": xin, "a_in": a_in}


def run(x, weight, bias, trace=False):
    nc = _get_nc()
    in_maps = [_prep_core(x, weight, bias, core) for core in range(N_CORES)]
    res = run_bass_kernel_spmd(nc, in_maps, list(range(N_CORES)), trace=trace)

    y = np.empty((B, DIM, T), dtype=np.float32)
    for core in range(N_CORES):
        yp = res.results[core]["yout"]                        # [PO, C, B*NJ]
        yc = yp.reshape(PO, C, B, NJ).transpose(2, 1, 3, 0)   # [B, C, j, m]
        y[:, core * C : (core + 1) * C, :] = yc.reshape(B, C, NJ * PO)[:, :, :T]
    return y, res


def kernel(x, weight, bias):
    y, _ = run(
        np.asarray(x, dtype=np.float32),
        np.asarray(weight, dtype=np.float32),
        np.asarray(bias, dtype=np.float32),
    )
    return y



# revision 2
# speedup vs baseline: 1.6277x; 1.6277x over previous
"""Depthwise causal Conv1d (K=16) for x:(4, 2048, 8192) f32 on 8 TRN2 NeuronCores.

Strategy (tensor-parallel over channels, no cross-core communication):
  - Each core owns 256 channels (2048 / 8) for all 4 batches.
  - The time axis is cut into overlapping 127-sample windows with stride 112
    (15-sample causal halo), placed on SBUF partitions 0..126 and
    time-REVERSED within each window; partition 127 carries a constant 1.0
    row.  The depthwise conv (+bias) of one channel is then a single
    banded-Toeplitz matmul on the TensorEngine:
        psum[m, (b,j)] = sum_p A[p, m] * X[p, (b,j)]
        A[p, m]   = w[126 - p - m]            for 111 <= p + m <= 126
        A[127, m] = bias_c                    (ones-row of X -> +bias)
        X[p, (b,j)] = x[b, c, 112*j + 111 - p]  (zero outside [0, T))
        X[127, (b,j)] = 1.0
        psum[m, (b,j)] = y[b, c, 112*j + m]     for m < 112
    The reversal makes every host/device access pattern a purely
    positive-stride AP; the halo removes any cross-window boundary matmul;
    112 = 7*16 output rows keep the store DMAs port-balanced.
  - All DRAM traffic is bf16 (inputs rounded on host, output upcast on
    host): the problem is HBM-bandwidth bound, and bf16 halves the bytes
    while staying ~0.4% rel err (budget is 2e-2).  Matmul runs in bf16
    (1 cyc/col vs 4 for f32), PSUM accumulates in f32.
  - Epilogue: PSUM(f32) -> SBUF(bf16) converting copies, two channels per
    instruction, alternating Vector / Scalar engines, then one large store
    per chunk.

The host does the sharding + window-layout transposes with numpy; the device
kernel sees only dense p-major arrays.
"""

import os
import sys

import numpy as np
from numpy.lib.stride_tricks import sliding_window_view

if "/opt/trn_rl_repo" not in sys.path:
    sys.path.insert(0, "/opt/trn_rl_repo")

import ml_dtypes

import concourse.bacc as bacc
import concourse.mybir as mybir
import concourse.tile as tile
from concourse.bass_utils import run_bass_kernel_spmd

F32 = mybir.dt.float32
BF16 = mybir.dt.bfloat16
NP_BF16 = np.dtype(ml_dtypes.bfloat16)
ACT_COPY = mybir.ActivationFunctionType.Copy

N_CORES = 8
B = 4             # batch
DIM = 2048        # channels
T = 8192          # time
K = 16            # conv taps
C = DIM // N_CORES    # channels per core = 256
PIN = 128         # matmul contraction rows (127 x-samples + ones row)
PO = 112          # outputs per window (= 127 - 15), multiple of 16
NJ = -(-T // PO)      # windows per (batch, channel) = 74
XC = B * NJ           # x / out cols per channel = 296
PSB = 512             # psum bank stride (f32 elems); channel pair at 0 / PSB
CH = 16           # channels per device chunk
NCHUNK = C // CH  # 16

_compiled_nc = None


def _build_kernel():
    nc = bacc.Bacc(None)

    xin = nc.declare_dram_parameter("xin", [PIN, C, XC], BF16, isOutput=False)
    a_in = nc.declare_dram_parameter("a_in", [PIN, C, PO], BF16, isOutput=False)
    yout = nc.declare_dram_parameter("yout", [PO, C, XC], BF16, isOutput=True)

    with tile.TileContext(nc) as tc:
        with (
            tc.tile_pool(name="xpool", bufs=4) as xpool,
            tc.tile_pool(name="apool", bufs=4) as apool,
            tc.tile_pool(name="opool", bufs=4) as opool,
            tc.tile_pool(name="psum", bufs=4, space="PSUM") as pspool,
        ):
            for chunk in range(NCHUNK):
                c0 = chunk * CH
                x_t = xpool.tile([PIN, CH * XC], BF16)
                a_t = apool.tile([PIN, CH * PO], BF16)
                o_t = opool.tile([PO, CH * XC], BF16)

                nc.gpsimd.dma_start(
                    out=x_t[:].rearrange("p (c j) -> p c j", c=CH),
                    in_=xin[:, c0 : c0 + CH, :],
                )
                a_eng = nc.scalar if chunk % 2 == 0 else nc.sync
                a_eng.dma_start(
                    out=a_t[:].rearrange("p (c m) -> p c m", c=CH),
                    in_=a_in[:, c0 : c0 + CH, :],
                )

                for g in range(CH // 2):
                    ps = pspool.tile([PO, 2 * PSB], F32)
                    for h in range(2):
                        i = 2 * g + h
                        nc.tensor.matmul(
                            ps[:, h * PSB : h * PSB + XC],
                            a_t[:, i * PO : (i + 1) * PO],
                            x_t[:, i * XC : (i + 1) * XC],
                            start=True,
                            stop=True,
                        )
                    # converting psum(f32) -> sbuf(bf16) copy, 2 ch per inst
                    src = ps[:].rearrange("p (g q) -> p g q", g=2)[:, :, 0:XC]
                    dst = o_t[:, 2 * g * XC : (2 * g + 2) * XC].rearrange(
                        "p (g q) -> p g q", g=2
                    )
                    if g % 2 == 0:
                        nc.vector.tensor_copy(dst, src)
                    else:
                        nc.scalar.activation(dst, src, ACT_COPY)

                s_eng = nc.sync if chunk % 2 == 0 else nc.scalar
                s_eng.dma_start(
                    out=yout[:, c0 : c0 + CH, :],
                    in_=o_t[:].rearrange("p (c j) -> p c j", c=CH),
                )

    nc.compile()
    return nc


def _get_nc():
    global _compiled_nc
    if _compiled_nc is None:
        _compiled_nc = _build_kernel()
    return _compiled_nc


def _prep_core(x, weight, bias, core):
    """Build the per-core input map (numpy only)."""
    cs = slice(core * C, (core + 1) * C)
    xs = x[:, cs, :]                       # [B, C, T]
    w = weight[cs, 0, :]                   # [C, K]
    bs = bias[cs]                          # [C]

    # X[p, c, (b, j)] = xpad[b, c, 112*j + 126 - p] for p < 127; X[127] = 1
    # xpad = [15 zeros] ++ x ++ [right pad]
    xpad = np.zeros((B, C, PO * (NJ - 1) + PIN - 1), dtype=np.float32)
    xpad[:, :, K - 1 : K - 1 + T] = xs
    sw = sliding_window_view(xpad, PIN - 1, axis=2)[:, :, :: PO, :]  # [B,C,NJ,127]
    xin = np.empty((PIN, C, B, NJ), dtype=np.float32)
    xin[0 : PIN - 1] = sw[:, :, :, ::-1].transpose(3, 1, 0, 2)
    xin[PIN - 1] = 1.0
    xin = np.ascontiguousarray(xin).reshape(PIN, C, XC).astype(NP_BF16)

    # A[p, m] = w[126 - p - m] for 111 <= p + m <= 126; A[127, m] = bias
    idx = np.arange(PIN - 1)[:, None] + np.arange(PO)[None, :]   # p + m
    amask = (idx >= 111) & (idx <= 126)
    aidx = np.clip(126 - idx, 0, K - 1)
    a_mat = np.where(amask[None], w[:, aidx], 0.0)               # [C, 127, PO]
    a_in = np.empty((PIN, C, PO), dtype=np.float32)
    a_in[0 : PIN - 1] = a_mat.transpose(1, 0, 2)
    a_in[PIN - 1] = bs[:, None]
    a_in = np.ascontiguousarray(a_in).astype(NP_BF16)

    return {"xin": xin, "a_in": a_in}


def run(x, weight, bias, trace=False):
    nc = _get_nc()
    in_maps = [_prep_core(x, weight, bias, core) for core in range(N_CORES)]
    res = run_bass_kernel_spmd(nc, in_maps, list(range(N_CORES)), trace=trace)

    y = np.empty((B, DIM, T), dtype=np.float32)
    for core in range(N_CORES):
        yp = res.results[core]["yout"].astype(np.float32)        # [PO, C, B*NJ]
        yc = yp.reshape(PO, C, B, NJ).transpose(2, 1, 3, 0)      # [B, C, j, m]
        y[:, core * C : (core + 1) * C, :] = yc.reshape(B, C, NJ * PO)[:, :, :T]
    return y, res


def kernel(x, weight, bias):
    y, _ = run(
        np.asarray(x, dtype=np.float32),
        np.asarray(weight, dtype=np.float32),
        np.asarray(bias, dtype=np.float32),
    )
    return y


# revision 5
# speedup vs baseline: 1.9192x; 1.1791x over previous
"""Depthwise causal Conv1d (K=16) for x:(4, 2048, 8192) f32 on 8 TRN2 NeuronCores.

Strategy (tensor-parallel over channels, no cross-core communication):
  - Each core owns 256 channels (2048 / 8) for all 4 batches.
  - The time axis is cut into overlapping 127-sample windows with stride 112
    (15-sample causal halo), placed on SBUF partitions 0..126 and
    time-REVERSED within each window; partition 127 carries a constant 1.0
    row.  The depthwise conv (+bias) of one channel is then a single
    banded-Toeplitz matmul on the TensorEngine:
        psum[m, (b,j)] = sum_p A[p, m] * X[p, (b,j)]
        A[p, m]   = w[126 - p - m]            for 111 <= p + m <= 126
        A[127, m] = bias_c                    (ones-row of X -> +bias)
        X[p, (b,j)] = x[b, c, 112*j + 111 - p]  (zero outside [0, T))
        X[127, (b,j)] = 1.0
        psum[m, (b,j)] = y[b, c, 112*j + m]     for m < 112
    The reversal makes every host/device access pattern a purely
    positive-stride AP; the halo removes any cross-window boundary matmul;
    112 = 7*16 output rows keep the store DMAs port-balanced.
  - All DRAM traffic is bf16 (inputs rounded on host, output upcast on
    host): the problem is HBM-bandwidth bound, and bf16 halves the bytes
    while staying ~0.4% rel err (budget is 2e-2).  Matmul runs in bf16
    (1 cyc/col vs 4 for f32), PSUM accumulates in f32.
  - Epilogue: PSUM(f32) -> SBUF(bf16) converting copies, two channels per
    instruction, alternating Vector / Scalar engines, then one large store
    per chunk.

The host does the sharding + window-layout transposes with numpy; the device
kernel sees only dense p-major arrays.
"""

import os
import sys

import numpy as np
from numpy.lib.stride_tricks import sliding_window_view

if "/opt/trn_rl_repo" not in sys.path:
    sys.path.insert(0, "/opt/trn_rl_repo")

import ml_dtypes

import concourse.bacc as bacc
import concourse.mybir as mybir
import concourse.tile as tile
from concourse.bass_utils import run_bass_kernel_spmd

F32 = mybir.dt.float32
BF16 = mybir.dt.bfloat16
NP_BF16 = np.dtype(ml_dtypes.bfloat16)
ACT_COPY = mybir.ActivationFunctionType.Copy

N_CORES = 8
B = 4             # batch
DIM = 2048        # channels
T = 8192          # time
K = 16            # conv taps
C = DIM // N_CORES    # channels per core = 256
PIN = 128         # matmul contraction rows (127 x-samples + ones row)
PO = 112          # outputs per window (= 127 - 15), multiple of 16
NJ = -(-T // PO)      # windows per (batch, channel) = 74
XC = B * NJ           # x / out cols per channel = 296
PSB = 512             # psum bank stride (f32 elems); channel pair at 0 / PSB
CH = 16           # channels per device chunk
NCHUNK = C // CH  # 16

_compiled_nc = None


def _build_kernel():
    nc = bacc.Bacc(None)

    xin = nc.declare_dram_parameter("xin", [PIN, C, XC], BF16, isOutput=False)
    a_in = nc.declare_dram_parameter("a_in", [PIN, C, PO], BF16, isOutput=False)
    yout = nc.declare_dram_parameter("yout", [PO, C, XC], BF16, isOutput=True)

    with tile.TileContext(nc) as tc:
        with (
            tc.tile_pool(name="xpool", bufs=6) as xpool,
            tc.tile_pool(name="apool", bufs=6) as apool,
            tc.tile_pool(name="opool", bufs=6) as opool,
            tc.tile_pool(name="psum", bufs=4, space="PSUM") as pspool,
        ):
            for chunk in range(NCHUNK):
                c0 = chunk * CH
                x_t = xpool.tile([PIN, CH * XC], BF16)
                a_t = apool.tile([PIN, CH * PO], BF16)
                o_t = opool.tile([PO, CH * XC], BF16)

                nc.gpsimd.dma_start(
                    out=x_t[:].rearrange("p (c j) -> p c j", c=CH),
                    in_=xin[:, c0 : c0 + CH, :],
                )
                nc.gpsimd.dma_start(
                    out=a_t[:].rearrange("p (c m) -> p c m", c=CH),
                    in_=a_in[:, c0 : c0 + CH, :],
                )

                for g in range(CH // 2):
                    ps = pspool.tile([PO, 2 * PSB], F32)
                    for h in range(2):
                        i = 2 * g + h
                        nc.tensor.matmul(
                            ps[:, h * PSB : h * PSB + XC],
                            a_t[:, i * PO : (i + 1) * PO],
                            x_t[:, i * XC : (i + 1) * XC],
                            start=True,
                            stop=True,
                        )
                    # converting psum(f32) -> sbuf(bf16) copy, 2 ch per inst
                    src = ps[:].rearrange("p (g q) -> p g q", g=2)[:, :, 0:XC]
                    dst = o_t[:, 2 * g * XC : (2 * g + 2) * XC].rearrange(
                        "p (g q) -> p g q", g=2
                    )
                    if g % 2 == 0:
                        nc.vector.tensor_copy(dst, src)
                    else:
                        nc.scalar.activation(dst, src, ACT_COPY)

                nc.sync.dma_start(
                    out=yout[:, c0 : c0 + CH, :],
                    in_=o_t[:].rearrange("p (c j) -> p c j", c=CH),
                )

    nc.compile()
    return nc


def _get_nc():
    global _compiled_nc
    if _compiled_nc is None:
        _compiled_nc = _build_kernel()
    return _compiled_nc


def _prep_core(x, weight, bias, core):
    """Build the per-core input map (numpy only)."""
    cs = slice(core * C, (core + 1) * C)
    xs = x[:, cs, :]                       # [B, C, T]
    w = weight[cs, 0, :]                   # [C, K]
    bs = bias[cs]                          # [C]

    # X[p, c, (b, j)] = xpad[b, c, 112*j + 126 - p] for p < 127; X[127] = 1
    # xpad = [15 zeros] ++ x ++ [right pad]
    xpad = np.zeros((B, C, PO * (NJ - 1) + PIN - 1), dtype=np.float32)
    xpad[:, :, K - 1 : K - 1 + T] = xs
    sw = sliding_window_view(xpad, PIN - 1, axis=2)[:, :, :: PO, :]  # [B,C,NJ,127]
    xin = np.empty((PIN, C, B, NJ), dtype=np.float32)
    xin[0 : PIN - 1] = sw[:, :, :, ::-1].transpose(3, 1, 0, 2)
    xin[PIN - 1] = 1.0
    xin = np.ascontiguousarray(xin).reshape(PIN, C, XC).astype(NP_BF16)

    # A[p, m] = w[126 - p - m] for 111 <= p + m <= 126; A[127, m] = bias
    idx = np.arange(PIN - 1)[:, None] + np.arange(PO)[None, :]   # p + m
    amask = (idx >= 111) & (idx <= 126)
    aidx = np.clip(126 - idx, 0, K - 1)
    a_mat = np.where(amask[None], w[:, aidx], 0.0)               # [C, 127, PO]
    a_in = np.empty((PIN, C, PO), dtype=np.float32)
    a_in[0 : PIN - 1] = a_mat.transpose(1, 0, 2)
    a_in[PIN - 1] = bs[:, None]
    a_in = np.ascontiguousarray(a_in).astype(NP_BF16)

    return {"xin": xin, "a_in": a_in}


def run(x, weight, bias, trace=False):
    nc = _get_nc()
    in_maps = [_prep_core(x, weight, bias, core) for core in range(N_CORES)]
    res = run_bass_kernel_spmd(nc, in_maps, list(range(N_CORES)), trace=trace)

    y = np.empty((B, DIM, T), dtype=np.float32)
    for core in range(N_CORES):
        yp = res.results[core]["yout"].astype(np.float32)        # [PO, C, B*NJ]
        yc = yp.reshape(PO, C, B, NJ).transpose(2, 1, 3, 0)      # [B, C, j, m]
        y[:, core * C : (core + 1) * C, :] = yc.reshape(B, C, NJ * PO)[:, :, :T]
    return y, res


def kernel(x, weight, bias):
    y, _ = run(
        np.asarray(x, dtype=np.float32),
        np.asarray(weight, dtype=np.float32),
        np.asarray(bias, dtype=np.float32),
    )
    return y
